# revision 7
# baseline (speedup 1.0000x reference)
"""Cross-attention kernel for Trainium2 (8 NeuronCores, SPMD data-parallel).

Problem: O = softmax(Q @ K^T) @ V with B=4, Lq=Lk=4096, D=64, fp32 (no
1/sqrt(d) scaling).

Sharding: 8 cores = 4 batches x 2 Lq-halves. Each core handles a
[2048, 64] Q shard against the full [4096, 64] K/V of its batch.
Independent outputs -> no collectives.

Per-core pipeline (one unit = one k-chunk of 128 keys x 1024 q):
  - ST[k, q] = matmul(lhsT=KT chunk [64,128] fp16, rhs=QT [64,512] fp16)
    -> PSUM [128, 1024] (2 banks, double-buffered).
  - P = exp(ST) -> bf16 SBUF [128, 1024], column-split across engines so
    neither exceeds the PE's 854ns/unit: the scalar engine does exact
    table exp on q-cols 0:512, the vector engine does a Schraudolph
    bit-trick exp (int16(A*s + B) reinterpreted as bf16 ~= e^s) on
    512:1024. The tensor engine must never stall: the timing model only
    grants the fast PE clock after ~3us of gap-free execution, so the
    program order is hand-pipelined (scores(u) issue before PVs(u-1))
    and output copies are deferred to the end.
  - OT[65, q] += matmul(lhsT=VA chunk [128, 65] bf16, rhs=PT [128,512]):
    VA = concat([V, ones], 1); rows 0..63 accumulate the unnormalized
    output, row 64 the softmax denominator. PSUM `start` zeroes a whole
    2KB bank, so each 512-col slice is one accumulation group per bank.
  - Warm-up matmuls on zeroed tiles run during the input-DMA head so the
    PE clock is already ramped when real work arrives; they target the
    OT banks, whose real groups clear them later.
  - Normalization (divide by row 64) happens on host after DMA-out,
    like the host-side transposes.
"""

import sys

for _p in ("/opt/trn_rl_repo", "/opt/pypackages"):
    if _p not in sys.path:
        sys.path.insert(0, _p)

from contextlib import ExitStack

import ml_dtypes
import numpy as np

import concourse.bacc as bacc
import concourse.mybir as mybir
import concourse.tile as tile
from concourse.bass_utils import run_bass_kernel_spmd

# Problem constants (hardcoded per contract).
B, LQ, LK, D = 4, 4096, 4096, 64
N_CORES = 8
LQ_SHARD = LQ * B // N_CORES  # 2048
KC = 128  # k-chunk (PV contraction tile)
NKC = LK // KC  # 32
QB = 1024  # q extent per unit
NQB = LQ_SHARD // QB  # 2
NU = NQB * NKC  # 64 units
SL = 512  # matmul moving-dim slice (one PSUM bank)

F32 = mybir.dt.float32
F16 = mybir.dt.float16
BF16 = mybir.dt.bfloat16
I16 = mybir.dt.int16

BF16NP = ml_dtypes.bfloat16

# Schraudolph constants for bf16: int16(A*s + B) bits viewed as bf16 ~ e^s.
SCH_A = float(128.0 / np.log(2.0))  # 184.664...
SCH_C = 8.0  # sawtooth centering shift
SCH_B = 128.0 * 127.0 - SCH_C + 0.5  # +0.5: float->int16 cast truncates

N_WARMUP_MM = 9  # PE clock ramp-up matmuls during the DMA head


def _build_program():
    nc = bacc.Bacc(
        "TRN2",
        target_bir_lowering=False,
        debug=False,
        num_devices=N_CORES,
    )
    qt_d = nc.declare_dram_parameter("QT", [D, LQ_SHARD], F16, isOutput=False)
    kt_d = nc.declare_dram_parameter("KT", [D, LK], F16, isOutput=False)
    va_d = nc.declare_dram_parameter("VA", [KC, NKC, D + 1], BF16, isOutput=False)
    o_d = nc.declare_dram_parameter("O", [D + 1, LQ_SHARD], F32, isOutput=True)

    with tile.TileContext(nc) as tc, ExitStack() as ctx:
        singles = ctx.enter_context(tc.tile_pool(name="singles", bufs=1))
        st_pool = ctx.enter_context(tc.tile_pool(name="st", bufs=2, space="PSUM"))
        ot_pool = ctx.enter_context(tc.tile_pool(name="ot", bufs=1, space="PSUM"))
        pt_pool = ctx.enter_context(tc.tile_pool(name="pt", bufs=3))

        # Preload the exp activation table while input DMAs run.
        warm = singles.tile([1, 2], F32)
        nc.vector.memset(warm[:, :], 0.0)
        nc.scalar.activation(
            out=warm[:, :], in_=warm[:, :],
            func=mybir.ActivationFunctionType.Exp,
        )

        wt = singles.tile([D, D + 1 + SL], F16)
        nc.vector.memset(wt[:, :], 0.0)

        # Input DMAs, earliest-needed first.
        qt = singles.tile([D, LQ_SHARD], F16, name="qt")
        kt = singles.tile([D, LK], F16, name="kt")
        va = singles.tile([KC, NKC, D + 1], BF16, name="va")
        KP = 4  # kt/va DMA pieces
        nc.sync.dma_start(out=qt[:, 0:QB], in_=qt_d[:, 0:QB])
        for h in range(KP):
            kw = LK // KP
            nc.sync.dma_start(
                out=kt[:, h * kw : (h + 1) * kw], in_=kt_d[:, h * kw : (h + 1) * kw]
            )
            cw = NKC // KP
            nc.sync.dma_start(
                out=va[:, h * cw : (h + 1) * cw, :],
                in_=va_d[:, h * cw : (h + 1) * cw, :],
            )
        nc.sync.dma_start(out=qt[:, QB:], in_=qt_d[:, QB:])

        ot = [ot_pool.tile([D + 1, QB], F32, name=f"ot{qh}") for qh in range(NQB)]

        # PE clock ramp-up on zeroed tiles (overlaps the DMA head). Each MM
        # is a complete start+stop group; the real OT groups re-zero later.
        for i in range(N_WARMUP_MM):
            nc.tensor.matmul(
                out=ot[i % NQB][:, 0:SL],
                lhsT=wt[:, 0 : D + 1],
                rhs=wt[:, D + 1 : D + 1 + SL],
                start=True,
                stop=True,
                skip_group_check=True,
            )

        # Software-pipelined main loop: unit u = (qh, c) = (u // NKC, u % NKC).
        # Emission order: scores(u), exp(u), PVs(u-1) -- so the PE queue
        # always holds ready score MMs while exp(u) runs on scalar/DVE.
        pts = [None] * NU

        def emit_scores_exp(u):
            qh, c = divmod(u, NKC)
            # Separate tiles per 512-col half so the scalar and vector exp
            # halves carry no write-write dependency and each PV slice waits
            # only on its own half.
            st_a = st_pool.tile([KC, SL], F32, tag="st_a")
            st_b = st_pool.tile([KC, SL], F32, tag="st_b")
            pt_s = pt_pool.tile([KC, SL], BF16, tag="pt_s")
            pt_d = pt_pool.tile([KC, SL], BF16, tag="pt_d")
            for s, st in ((0, st_a), (1, st_b)):
                nc.tensor.matmul(
                    out=st[:, :],
                    lhsT=kt[:, c * KC : (c + 1) * KC],
                    rhs=qt[:, qh * QB + s * SL : qh * QB + (s + 1) * SL],
                    start=True,
                    stop=True,
                )
            nc.scalar.activation(
                out=pt_s[:, :],
                in_=st_a[:, :],
                func=mybir.ActivationFunctionType.Exp,
            )
            nc.vector.tensor_scalar(
                pt_d[:, :].bitcast(I16),
                st_b[:, :],
                SCH_A,
                SCH_B,
                mybir.AluOpType.mult,
                mybir.AluOpType.add,
            )
            pts[u] = (pt_s, pt_d)

        def emit_pv(u):
            qh, c = divmod(u, NKC)
            for s in range(2):
                nc.tensor.matmul(
                    out=ot[qh][:, s * SL : (s + 1) * SL],
                    lhsT=va[:, c, :],
                    rhs=pts[u][s][:, :],
                    start=(c == 0),
                    stop=(c == NKC - 1),
                    skip_group_check=True,
                )

        emit_scores_exp(0)
        emit_scores_exp(1)
        for u in range(2, NU):
            emit_scores_exp(u)
            emit_pv(u - 2)
        emit_pv(NU - 2)
        emit_pv(NU - 1)

        # Output (normalization on host): copy PSUM->SBUF on both free
        # engines in parallel, then DMA out.
        ob0 = singles.tile([D + 1, QB], F32)
        ob1 = singles.tile([D + 1, QB], F32)
        nc.scalar.activation(
            out=ob0[:, :], in_=ot[0][:, :],
            func=mybir.ActivationFunctionType.Copy,
        )
        nc.vector.tensor_copy(ob1[:, :], ot[1][:, :])
        nc.sync.dma_start(out=o_d[:, 0:QB], in_=ob0[:, :])
        nc.sync.dma_start(out=o_d[:, QB:], in_=ob1[:, :])

    nc.finalize()
    return nc


_PROGRAM_CACHE = {}


def _get_program():
    if "nc" not in _PROGRAM_CACHE:
        _PROGRAM_CACHE["nc"] = _build_program()
    return _PROGRAM_CACHE["nc"]


def _make_in_maps(Q, K, V):
    Q = np.asarray(Q, dtype=np.float32)
    K = np.asarray(K, dtype=np.float32)
    V = np.asarray(V, dtype=np.float32)
    in_maps = []
    ones = np.ones((LK, 1), dtype=np.float32)
    for core in range(N_CORES):
        b, half = core // 2, core % 2
        q_shard = Q[b, half * LQ_SHARD : (half + 1) * LQ_SHARD, :]  # [2048, 64]
        qt = np.ascontiguousarray(q_shard.T).astype(np.float16)  # [64, 2048]
        kt = np.ascontiguousarray(K[b].T).astype(np.float16)  # [64, 4096]
        # VA[p, c, d] = concat([V, 1])[c*128 + p, d]
        va = np.ascontiguousarray(
            np.concatenate([V[b], ones], axis=1)
            .reshape(NKC, KC, D + 1)
            .swapaxes(0, 1)
        ).astype(BF16NP)
        in_maps.append({"QT": qt, "KT": kt, "VA": va})
    return in_maps


def _run(Q, K, V, trace=False, **spmd_kwargs):
    nc = _get_program()
    in_maps = _make_in_maps(Q, K, V)
    res = run_bass_kernel_spmd(
        nc, in_maps, list(range(N_CORES)), trace=trace, **spmd_kwargs
    )
    out = np.empty((B, LQ, D), dtype=np.float32)
    for core in range(N_CORES):
        b, half = core // 2, core % 2
        o = res.results[core]["O"]  # [65, 2048]
        shard = (o[0:D, :] / o[D : D + 1, :]).T  # [2048, 64]
        out[b, half * LQ_SHARD : (half + 1) * LQ_SHARD, :] = shard
    return out, res


def kernel(Q, K, V):
    out, _ = _run(Q, K, V, trace=False)
    return out
